# revision 20
# baseline (speedup 1.0000x reference)
"""Trainium2 Bass kernel for nn_AgnosticResidualInteractionBlock (GNN message passing).

Strategy (8 NeuronCores, receiver-node graph partition; all constant scales and
the pre-message linear W_pre are folded into host-side weights):
  - Host: pack nodes into 16-receiver *windows* balanced by degree so each
    window owns <=256 incoming edges (2 chunks of 128 edge slots); pre-gather
    raw sender features into a per-core bf16 edge stream (the device never
    gathers); fold W_pre @ W_post into four combined 128x128 matrices so the
    device scatters RAW features and applies one output linear per block.
  - Device (SPMD, one Bass program on 8 cores), per 128-receiver tile:
      radial MLP (packed two 1024-edge halves per 128-partition PSUM tile) ->
      per-edge scalar table Q; per 128-edge chunk: one is_equal builds the
      8-copy one-hot slab, one broadcast multiply scales it by Q, and five
      matmuls scatter-accumulate [channel x (window,block,slot)] into PSUM;
      post-message linear with the folded weights; species linear (sc) runs
      as a bf16 prologue.
  - Host: map receiver slots back to node ids, cast outputs to f32.
"""
import sys
import numpy as np

try:
    import concourse.bacc as bacc
except ImportError:  # pragma: no cover
    sys.path.insert(0, "/opt/trn_rl_repo")
    import concourse.bacc as bacc

import ml_dtypes

import concourse.bass as bass
import concourse.mybir as mybir
import concourse.tile as tile
from concourse.bass_utils import run_bass_kernel_spmd

BF16 = ml_dtypes.bfloat16
N, C, E, S = 16384, 128, 262144, 10
NCORES = 8
AVG = 16.0
INV_C = 1.0 / np.sqrt(C)
INV2C = 1.0 / np.sqrt(2 * C)
INV_SQRT3 = 1.0 / np.sqrt(3.0)
W = 16                  # receivers per window
WCAP = 256              # edge slots per window (2 chunks)

_CACHE = {}


def _pack_windows(deg, nwc):
    """Greedy best-fit-decreasing: nodes -> (core, window) with <=16 nodes and
    <=256 edges per window. Returns (core_of, win_of, slot_of) or None."""
    nwin = NCORES * nwc
    cap = np.full(nwin, WCAP, np.int64)
    slots = np.full(nwin, W, np.int64)
    core_of = np.zeros(N, np.int64)
    win_of = np.zeros(N, np.int64)
    slot_of = np.zeros(N, np.int64)
    order = np.argsort(-deg, kind="stable")
    eligible = np.ones(nwin, bool)
    for n in order:
        d = deg[n]
        c = np.where(eligible, cap, -1)
        w = int(np.argmax(c))
        if c[w] < d:
            return None
        cap[w] -= d
        slots[w] -= 1
        slot_of[n] = W - 1 - slots[w]
        if slots[w] == 0:
            eligible[w] = False
        core_of[n] = w // nwc
        win_of[n] = w % nwc
    return core_of, win_of, slot_of


def _host_prep(node_specie, node_feats, edge_attrs, edge_feats, senders, receivers,
               W_sc0, W_sc1, W_pre0, W_pre1, W_mlp1, W_mlp2, W_mlp3, W_post0, W_post1):
    senders = np.asarray(senders).astype(np.int64)
    receivers = np.asarray(receivers).astype(np.int64)
    node_specie = np.asarray(node_specie).astype(np.int64)
    node_feats = np.asarray(node_feats, dtype=np.float32)
    edge_attrs = np.asarray(edge_attrs, dtype=np.float32)
    edge_feats = np.asarray(edge_feats, dtype=np.float32)

    # ---- window packing (receiver partition balanced by degree) ------------
    deg = np.bincount(receivers, minlength=N)
    nwc = 131
    while True:
        res = _pack_windows(deg, nwc)
        if res is not None:
            break
        nwc += 2
    core_of, win_of, slot_of = res
    NT = -(-nwc // 8)            # tiles per core
    NWC = NT * 8                 # padded windows per core
    NCH = NT * 16                # chunks per core
    EPADP = NCH * 128            # edge slots per core

    # ---- edge placement into per-core slot streams -------------------------
    er_core = core_of[receivers]
    er_win = win_of[receivers]
    gw = er_core * NWC + er_win
    eorder = np.argsort(gw, kind="stable")
    gw_s = gw[eorder]
    within = np.arange(E) - np.searchsorted(gw_s, gw_s)  # index within window
    slot_global = gw_s * WCAP + within                   # global slot id
    # per-core slot arrays
    send_slot = np.full(NCORES * EPADP, -1, np.int64)
    rloc_slot = np.full(NCORES * EPADP, -1.0, np.float32)
    a_slot = np.zeros((NCORES * EPADP, 3), np.float32)
    ef_slot = np.zeros((NCORES * EPADP, 8), np.float32)
    send_slot[slot_global] = senders[eorder]
    rloc_slot[slot_global] = slot_of[receivers[eorder]].astype(np.float32)
    a_slot[slot_global] = edge_attrs[eorder][:, 1:4]
    ef_slot[slot_global] = edge_feats[eorder]

    # ---- species permutation per core (sc stage) ---------------------------
    nodes_of_core = [np.nonzero(core_of == k)[0] for k in range(NCORES)]
    spec_counts = np.zeros((NCORES, S), np.int64)
    for k in range(NCORES):
        spec_counts[k] = np.bincount(node_specie[nodes_of_core[k]], minlength=S)
    tiles_per_spec = np.maximum(1, -(-spec_counts.max(axis=0) // 128))
    NSLOT = int(tiles_per_spec.sum() * 128)
    spec_tile_off = np.concatenate([[0], np.cumsum(tiles_per_spec)])[:-1] * 128
    species_of_tile = np.repeat(np.arange(S), tiles_per_spec)
    node_of_scslot = -np.ones((NCORES, NSLOT), np.int64)   # global node ids
    for k in range(NCORES):
        nk = nodes_of_core[k]
        sp = node_specie[nk]
        for s in range(S):
            g = nk[sp == s]
            node_of_scslot[k, spec_tile_off[s] + np.arange(len(g))] = g

    # ---- weights (scales folded) -------------------------------------------
    sc_post = INV2C / AVG
    Wp0 = np.asarray(W_pre0, np.float64) * INV_C
    Wp1 = np.asarray(W_pre1, np.float64) * INV_C
    Wo0 = np.asarray(W_post0, np.float64) * sc_post
    Wo1 = np.asarray(W_post1, np.float64) * sc_post
    A = Wp0 @ Wo0[:, 0, :]
    B = (Wp1 @ Wo0[:, 1, :]) * INV_SQRT3
    Cm = Wp1 @ Wo1[:, 0, :]
    D = Wp0 @ Wo1[:, 1, :]
    wcomb = np.concatenate([A, B, Cm, D], axis=1).astype(np.float32).astype(BF16)
    wsc0T = ((np.asarray(W_sc0, np.float32) * INV_C)
             .transpose(1, 0, 2).reshape(128, S * 128)).astype(BF16)
    wsc1T = ((np.asarray(W_sc1, np.float32) * INV_C)
             .transpose(1, 0, 2).reshape(128, S * 128)).astype(BF16)
    wm1 = (np.asarray(W_mlp1, np.float32) / np.sqrt(8.0)).astype(BF16)
    wm2d = np.zeros((128, 64), np.float32)
    wm2d[0:64] = np.asarray(W_mlp2, np.float32) * 0.125
    wm2d[64:128] = wm2d[0:64]
    wm2d = wm2d.astype(BF16)
    wm3d = np.zeros((128, 4), np.float32)
    wm3d[0:64] = np.asarray(W_mlp3, np.float32) * 0.125
    wm3d[64:128] = wm3d[0:64]
    wm3d = wm3d.astype(BF16)
    iota16 = np.tile(np.arange(W, dtype=np.float32), (128, 8)).astype(BF16)

    # ---- per-core device arrays -------------------------------------------
    nfb2 = np.ascontiguousarray(
        node_feats.transpose(0, 2, 1).reshape(N, 512)).astype(BF16)
    nfb3 = np.concatenate([nfb2, np.zeros((1, 512), BF16)], axis=0)
    per_core = []
    for k in range(NCORES):
        sl = slice(k * EPADP, (k + 1) * EPADP)
        snd = send_slot[sl]
        rows = nfb3[snd]                       # [-1] -> zero row
        rows[snd < 0] = 0
        fgS = np.ascontiguousarray(
            rows.reshape(NCH, 128, 512).transpose(1, 0, 2).reshape(128, NCH * 512))
        rlocT = np.ascontiguousarray(rloc_slot[sl].reshape(NCH, 128).T)
        aT = np.ascontiguousarray(
            a_slot[sl].reshape(NCH, 128, 3).transpose(1, 0, 2)
            .reshape(128, NCH * 3)).astype(BF16)
        efT = np.ascontiguousarray(ef_slot[sl].T).astype(BF16)
        nfT = np.zeros((4, 128, NSLOT), np.float32)
        valid = node_of_scslot[k] >= 0
        nfT[:, :, valid] = node_feats[node_of_scslot[k][valid]].transpose(2, 1, 0)
        per_core.append(dict(fgS=fgS, rlocT=rlocT, aT=aT, efT=efT,
                             nfT=nfT.astype(BF16)))

    shared = dict(wsc0T=wsc0T, wsc1T=wsc1T, wcomb=wcomb, wm1=wm1, wm2d=wm2d,
                  wm3d=wm3d, iota16=iota16)
    meta = dict(NT=NT, NCH=NCH, NSLOT=NSLOT,
                species_of_tile=tuple(int(x) for x in species_of_tile))
    unshard = dict(core_of=core_of, win_of=win_of, slot_of=slot_of,
                   node_of_scslot=node_of_scslot, NT=NT, NSLOT=NSLOT)
    return meta, per_core, shared, unshard


def _dep(later, earlier):
    tile.add_dep_helper(later.ins, earlier.ins, sync=False, reason="order")


_BUILD_STAGE = "full"   # debug knob: "sc" | "mlp" | "scatter" | "full"


def _build(meta):
    NT, NCH, NSLOT = meta["NT"], meta["NCH"], meta["NSLOT"]
    species_of_tile = meta["species_of_tile"]
    NGT = NSLOT // 128
    EPADP = NCH * 128
    f32, bf16 = mybir.dt.float32, mybir.dt.bfloat16

    nc = bacc.Bacc("TRN2", target_bir_lowering=False)
    fgS = nc.dram_tensor("fgS", [128, NCH * 512], bf16, kind="ExternalInput")
    rlocT = nc.dram_tensor("rlocT", [128, NCH], f32, kind="ExternalInput")
    aT = nc.dram_tensor("aT", [128, NCH * 3], bf16, kind="ExternalInput")
    efT = nc.dram_tensor("efT", [8, EPADP], bf16, kind="ExternalInput")
    nfT = nc.dram_tensor("nfT", [4, 128, NSLOT], bf16, kind="ExternalInput")
    wsc0T = nc.dram_tensor("wsc0T", [128, S * 128], bf16, kind="ExternalInput")
    wsc1T = nc.dram_tensor("wsc1T", [128, S * 128], bf16, kind="ExternalInput")
    wcomb = nc.dram_tensor("wcomb", [128, 512], bf16, kind="ExternalInput")
    wm1 = nc.dram_tensor("wm1", [8, 64], bf16, kind="ExternalInput")
    wm2d = nc.dram_tensor("wm2d", [128, 64], bf16, kind="ExternalInput")
    wm3d = nc.dram_tensor("wm3d", [128, 4], bf16, kind="ExternalInput")
    iota16 = nc.dram_tensor("iota16", [128, 128], bf16, kind="ExternalInput")
    sc_out = nc.dram_tensor("sc_out", [NSLOT, 512], bf16, kind="ExternalOutput")
    nout = nc.dram_tensor("nout", [NT * 128, 512], bf16, kind="ExternalOutput")

    with tile.TileContext(nc) as tc:
        with tc.tile_pool(name="cst", bufs=1) as cst:
            iota_sb = cst.tile([128, 128], bf16)
            nc.sync.dma_start(iota_sb[:], iota16[:])
            wcomb_sb = cst.tile([128, 512], bf16)
            nc.sync.dma_start(wcomb_sb[:], wcomb[:])
            wm1_sb = cst.tile([8, 64], bf16)
            nc.sync.dma_start(wm1_sb[:], wm1[:])
            wm2_sb = cst.tile([128, 64], bf16)
            nc.sync.dma_start(wm2_sb[:], wm2d[:])
            wm3_sb = cst.tile([128, 4], bf16)
            nc.sync.dma_start(wm3_sb[:], wm3d[:])
            rloc_sb = cst.tile([128, NCH], f32)
            nc.sync.dma_start(rloc_sb[:], rlocT[:])
            aT_sb = cst.tile([128, NCH * 3], bf16)
            nc.sync.dma_start(aT_sb[:], aT[:])
            ef_sb = cst.tile([8, EPADP], bf16)
            nc.sync.dma_start(ef_sb[:], efT[:])
            q8 = cst.tile([128, 8, NCH], bf16)

            # ---- sc: per-species linear prologue (bf16) --------------------
            with tc.tile_pool(name="sa", bufs=1) as sa, \
                 tc.tile_pool(name="sa2", bufs=3) as sa2, \
                 tc.tile_pool(name="psA", bufs=2, space="PSUM") as psA:
                wsc0_sb = sa.tile([128, S * 128], bf16)
                nc.sync.dma_start(wsc0_sb[:], wsc0T[:])
                wsc1_sb = sa.tile([128, S * 128], bf16)
                nc.sync.dma_start(wsc1_sb[:], wsc1T[:])
                nfc = []
                for comp in range(4):
                    t = sa.tile([128, NSLOT], bf16, name=f"nfc{comp}")
                    nc.sync.dma_start(t[:], nfT[comp, :, :])
                    nfc.append(t)
                for gt in range(NGT):
                    sp = species_of_tile[gt]
                    ps_sc = psA.tile([128, 512], f32, tag="ps_sc", space="PSUM")
                    chain = []
                    for comp in range(4):
                        lhsT = nfc[comp][:, gt * 128:(gt + 1) * 128]
                        wsc = (wsc0_sb if comp == 0 else wsc1_sb)[:, sp * 128:(sp + 1) * 128]
                        chain.append(nc.tensor.matmul(
                            ps_sc[:, comp * 128:(comp + 1) * 128],
                            lhsT=lhsT, rhs=wsc, start=True, stop=True))
                    for x, y in zip(chain, chain[1:]):
                        _dep(y, x)
                    sc_sb = sa2.tile([128, 512], bf16, tag="sc_sb")
                    if gt % 2 == 0:
                        nc.vector.tensor_copy(sc_sb[:], ps_sc[:])
                    else:
                        nc.scalar.copy(sc_sb[:], ps_sc[:])
                    nc.sync.dma_start(sc_out[gt * 128:(gt + 1) * 128, :], sc_sb[:])

            # ---- main loop: MLP+Q then scatter per 128-receiver tile -------
            with tc.tile_pool(name="fg", bufs=5) as fgp, \
                 tc.tile_pool(name="mlp", bufs=2) as mlpp, \
                 tc.tile_pool(name="hh", bufs=6) as hhp, \
                 tc.tile_pool(name="ev", bufs=2) as evp, \
                 tc.tile_pool(name="psm", bufs=1, space="PSUM") as psm, \
                 tc.tile_pool(name="psagg", bufs=2, space="PSUM") as psagg, \
                 tc.tile_pool(name="pso", bufs=1, space="PSUM") as pso:
                for t in range(NT):
                    if _BUILD_STAGE == "sc":
                        break
                    e0 = t * 2048          # first edge slot of tile
                    c0 = t * 16            # first chunk of tile
                    fg = fgp.tile([128, 16, 512], bf16, tag="fg")
                    nc.sync.dma_start(
                        fg[:].rearrange("p a b -> p (a b)"),
                        fgS[:, c0 * 512:(c0 + 16) * 512])
                    if _BUILD_STAGE == "fg":
                        continue
                    # -- radial MLP for this tile's 2048 edges (2x1024 packed)
                    ps_h = psm.tile([128, 1024], f32, tag="ps_h", space="PSUM")
                    mm = []
                    for half in range(2):
                        for bk in range(2):
                            eo = e0 + half * 1024 + bk * 512
                            mm.append(nc.tensor.matmul(
                                ps_h[half * 64:(half + 1) * 64,
                                     bk * 512:(bk + 1) * 512],
                                lhsT=wm1_sb[:], rhs=ef_sb[:, eo:eo + 512],
                                start=True, stop=True))
                    for x, y in zip(mm, mm[1:]):
                        _dep(y, x)
                    h1 = mlpp.tile([128, 1024], bf16, tag="h1")
                    nc.scalar.activation(h1[:], ps_h[:],
                                         mybir.ActivationFunctionType.Silu)
                    if _BUILD_STAGE == "l1":
                        continue
                    ps_h2 = psm.tile([128, 1024], f32, tag="ps_h", space="PSUM")
                    mm = []
                    for half in range(2):
                        hw = wm2_sb[half * 64:(half + 1) * 64, :]
                        for bk in range(2):
                            mm.append(nc.tensor.matmul(
                                ps_h2[half * 64:(half + 1) * 64,
                                      bk * 512:(bk + 1) * 512],
                                lhsT=hw,
                                rhs=h1[half * 64:(half + 1) * 64,
                                       bk * 512:(bk + 1) * 512],
                                start=True, stop=True))
                    for x, y in zip(mm, mm[1:]):
                        _dep(y, x)
                    h2 = mlpp.tile([128, 1024], bf16, tag="h2")
                    nc.scalar.activation(h2[:], ps_h2[:],
                                         mybir.ActivationFunctionType.Silu)
                    if _BUILD_STAGE == "l2":
                        continue
                    ps_mix = psm.tile([128, 64], f32, tag="ps_mix", space="PSUM")
                    mm = []
                    for j in range(16):
                        half, jj = j // 8, j % 8
                        mm.append(nc.tensor.matmul(
                            ps_mix[:, j * 4:j * 4 + 4],
                            lhsT=h2[half * 64:(half + 1) * 64,
                                    jj * 128:(jj + 1) * 128],
                            rhs=wm3_sb[half * 64:(half + 1) * 64, :],
                            start=True, stop=True))
                    for x, y in zip(mm, mm[1:]):
                        _dep(y, x)
                    mix = mlpp.tile([128, 16, 4], bf16, tag="mix")
                    nc.vector.tensor_copy(
                        mix[:].rearrange("p a b -> p (a b)"), ps_mix[:])
                    if _BUILD_STAGE == "mix":
                        continue
                    # -- Q table for chunks c0..c0+16
                    # blocks: [mix0, a1*m3, a2*m3, a3*m3, a2*m1, mix2, a1*m1, a3*m1]
                    a3v = aT_sb[:, c0 * 3:(c0 + 16) * 3].rearrange(
                        "p (c k) -> p k c", k=3)
                    nc.vector.tensor_copy(q8[:, 0, c0:c0 + 16], mix[:, :, 0])
                    nc.vector.tensor_copy(q8[:, 5, c0:c0 + 16], mix[:, :, 2])
                    if _BUILD_STAGE == "q1":
                        continue
                    nc.vector.tensor_tensor(
                        out=q8[:, 1:4, c0:c0 + 16], in0=a3v[:, :, :],
                        in1=mix[:, :, 3:4].rearrange("p c 1 -> p 1 c")
                            .to_broadcast([128, 3, 16]),
                        op=mybir.AluOpType.mult)
                    if _BUILD_STAGE == "q2":
                        continue
                    for blk, kk in ((4, 1), (6, 0), (7, 2)):
                        nc.vector.tensor_tensor(
                            out=q8[:, blk, c0:c0 + 16], in0=a3v[:, kk, :],
                            in1=mix[:, :, 1], op=mybir.AluOpType.mult)
                    # -- scatter-accumulate 16 chunks into agg PSUM
                    if _BUILD_STAGE == "mlp":
                        continue
                    agg = psagg.tile([128, 1024], f32, tag="agg", space="PSUM")
                    bank_mms = [[], []]
                    for j in range(16):
                        ch = c0 + j
                        w = j // 2
                        bank = w // 4
                        oh = hhp.tile([128, 128], bf16, tag="oh")
                        nc.vector.tensor_scalar(
                            oh[:], iota_sb[:], rloc_sb[:, ch:ch + 1], None,
                            mybir.AluOpType.is_equal)
                        hall = hhp.tile([128, 8, 16], bf16, tag="hall")
                        nc.vector.tensor_tensor(
                            out=hall[:], in0=oh[:].rearrange("p (a b) -> p a b", b=16),
                            in1=q8[:, :, ch:ch + 1].to_broadcast([128, 8, 16]),
                            op=mybir.AluOpType.mult)
                        hflat = hall[:].rearrange("p a b -> p (a b)")
                        wb = w * 128
                        # each mm's PSUM region must be uniformly fresh or
                        # uniformly accumulated (pending-zero granularity),
                        # so vy's dot and m3 writes stay separate matmuls
                        for (comp, col, ncol, lo) in (
                            (0, wb + 0, 64, 0),
                            (1, wb + 64, 32, 80),
                            (2, wb + 80, 16, 64),
                            (2, wb + 96, 16, 80),
                            (3, wb + 80, 16, 112),
                            (3, wb + 112, 16, 80),
                        ):
                            mmi = nc.tensor.matmul(
                                agg[:, col:col + ncol],
                                lhsT=fg[:, j, comp * 128:(comp + 1) * 128],
                                rhs=hflat[:, lo:lo + ncol],
                                start=(len(bank_mms[bank]) == 0), stop=False)
                            bank_mms[bank].append(mmi)
                    for bank in range(2):
                        mms = bank_mms[bank]
                        mms[-1].ins.stop_tensor_calc = True
                        for m in mms[1:]:
                            _dep(m, mms[0])
                        for m in mms[:-1]:
                            _dep(mms[-1], m)
                    # evict with (window, block, slot) -> (block, window, slot)
                    # permute so each postmp lhsT block is one contiguous run
                    agg_sb = evp.tile([128, 1024], bf16, tag="agg_sb")
                    nc.scalar.copy(
                        agg_sb[:].rearrange("p (b a c) -> p b a c", b=8, a=8, c=16),
                        agg[:].rearrange("p (a b c) -> p b a c", a=8, b=8, c=16))
                    # -- postmp with folded weights: blocks
                    # [s0, sA, sB, sC, m2, dot, m3, m4] x [A,B,C,D]
                    if _BUILD_STAGE == "scatter":
                        nc.sync.dma_start(nout[t * 128:(t + 1) * 128, :],
                                          agg_sb[:, 0:512])
                        continue
                    o_ps = pso.tile([128, 512], f32, tag="o_ps", space="PSUM")
                    och = []
                    for (ocol, blk, wblk, st) in (
                        (0, 0, 0, True), (0, 5, 1, False),    # o_s = s0@A + dot@B
                        (128, 4, 2, True), (128, 1, 3, False),  # o_vx = m2@C+sA@D
                        (256, 6, 2, True), (256, 2, 3, False),  # o_vy = m3@C+sB@D
                        (384, 7, 2, True), (384, 3, 3, False),  # o_vz = m4@C+sC@D
                    ):
                        och.append(nc.tensor.matmul(
                            o_ps[:, ocol:ocol + 128],
                            lhsT=agg_sb[:, blk * 128:(blk + 1) * 128],
                            rhs=wcomb_sb[:, wblk * 128:(wblk + 1) * 128],
                            start=st, stop=not st))
                    for x, y in zip(och, och[1:]):
                        _dep(y, x)
                    out_sb = evp.tile([128, 512], bf16, tag="out_sb")
                    nc.scalar.copy(
                        out_sb[:].rearrange("p (d c) -> p c d", c=4),
                        o_ps[:].rearrange("p (c d) -> p c d", c=4))
                    nc.sync.dma_start(nout[t * 128:(t + 1) * 128, :], out_sb[:])

    nc.compile()
    return nc


def kernel(**inputs):
    meta, per_core, shared, unshard = _host_prep(**inputs)
    key = (meta["NT"], meta["NCH"], meta["NSLOT"], meta["species_of_tile"])
    if key not in _CACHE:
        _CACHE[key] = _build(meta)
    nc = _CACHE[key]
    in_maps = [dict(pc, **shared) for pc in per_core]
    res = run_bass_kernel_spmd(nc, in_maps, core_ids=list(range(NCORES)))

    NT, NSLOT = unshard["NT"], unshard["NSLOT"]
    core_of, win_of, slot_of = (unshard["core_of"], unshard["win_of"],
                                unshard["slot_of"])
    node_of_scslot = unshard["node_of_scslot"]
    node_out = np.zeros((N, 128, 4), np.float32)
    sc = np.zeros((N, 128, 4), np.float32)
    rows_all = win_of * W + slot_of          # per-core row in nout
    for k in range(NCORES):
        nk = np.nonzero(core_of == k)[0]
        no_k = np.asarray(res.results[k]["nout"], dtype=np.float32)
        node_out[nk] = no_k[rows_all[nk]].reshape(-1, 128, 4)
        valid = node_of_scslot[k] >= 0
        rows = np.asarray(res.results[k]["sc_out"], dtype=np.float32)[valid]
        sc[node_of_scslot[k][valid]] = rows.reshape(-1, 4, 128).transpose(0, 2, 1)
    return node_out, sc
